# revision 32
# baseline (speedup 1.0000x reference)
"""Trainium2 Bass kernel for nn_Attention_layer_41429254537559.

Reference math:
    img_score = einsum('nld,d->nl', img, w)          # [N, L]
    q_score   = einsum('ntd,d->nt', qes, w)          # [N, T]
    logits    = q_score[:,:,None] + img_score[:,None,:]
    att       = softmax(logits, axis=2)              # over L
    out       = qes + einsum('ntl,nld->ntd', att, img)

q_score is constant along the softmax axis, so it cancels:
    a[n,:]  = softmax(img @ w)        # [N, L]
    c[n,:]  = a[n,:] @ img[n]         # [N, D]
    out     = qes + c[:,None,:]

Distribution: data-parallel over N across 8 cores (2 batch elements per core),
no collectives.

Per-core dataflow (v5; n_loc = 2 batches, L = 196 = 2x98 chunks, D = 1024):
  - Inputs host-cast to bf16, split across the SP and ACT HWDGE queues so
    the ~650ns/DMA sequencer cadence cannot gap the shared DMA bus; bus
    order = consumption order: w (replicated [98,1024]), b0c0, b0c1,
    b1c0, b1c1, then qes relaid as [128,512].
  - Scores: chunk b0c0 is a DVE bf16 2x tensor-mul reduced on ACT
    (activation accum_out); the other three chunks are fused DVE
    affine_mul_reduce (1127ns each; no DVE fast mode exists for them).
    b0's exp is one 2-wide ACT op; b1's exps are free-size-1 ACT ops
    pinned late via tile_wait_until so the scheduler cannot commit them
    ahead of b0's exp (head-of-line blocking in the 4-deep ACT wait
    queue otherwise stalls batch 0 by ~1us).
  - The ENTIRE per-core output lives in ONE PSUM bank psA [128,512] f32:
    partitions 0-31 = b0 cols 0-511, 32-63 = b1 cols 0-511, 64-95 = b0
    cols 512-1023, 96-127 = b1 cols 512-1023, via partition-offset
    (tile_position) matmuls with unnormalized e broadcast as lhsT - four
    independent accumulation regions in one bank.
  - Softmax denominators: per-batch e-column adds on DVE, then
    gpsimd.partition_all_reduce gives S replicated across 128 partitions;
    tiny DVE reciprocals assemble the per-partition 1/S pattern
    (recip128) off the critical path.
  - 12 PE warmup matmuls keep the PE p-state ramped into the
    weighted-sum matmuls (an idle gap resets the ramp model).
  - Epilogue: one DVE scalar_tensor_tensor (psA*(1/S) + qes128) into the
    staging tile.
  - Output write: a kv_writeback PREPARED early on the Pool SWDGE ring
    (descriptor generation hides behind the score phase; batch=1,
    d_head=128, ncn=512 maps stage partition p to output row p exactly),
    fired by one trigger_dma ordered after the stt through a Pool token
    copy with a RAW on the stage plus an explicit no-sync edge.  This
    replaces ~1.3us of HWDGE issue latency in the tail.  Validated on
    hardware: 8/8 cores produce rel err 2.4e-3.  The host un-permutes
    the four 32-row blocks.
"""

import numpy as np

N_CORES = 8
N, L, D, T = 16, 196, 1024, 32
NL = N // N_CORES  # batch elements per core
NC = 2  # l-chunks per batch element
LC = L // NC  # 98 rows per chunk
H = 512  # output column half

_CACHE = {}


def _build_nc():
    import concourse.tile as tile
    from concourse import bacc, bass_isa, mybir

    f32 = mybir.dt.float32
    bf16 = mybir.dt.bfloat16
    i16 = mybir.dt.int16
    Alu = mybir.AluOpType
    Act = mybir.ActivationFunctionType

    nc = bacc.Bacc(None, target_bir_lowering=False)

    # chunk order on the wire: A=b0c0, B=b0c1, C=b1c0, D=b1c1
    img = nc.dram_tensor("img", [NL, NC, LC, D], bf16, kind="ExternalInput")
    qes128 = nc.dram_tensor("qes128", [128, H], bf16, kind="ExternalInput")
    wb = nc.dram_tensor("wb", [LC, D], bf16, kind="ExternalInput")
    # kv_writeback-shaped output [batch=1, d_head=128, dho=1, n_ctx=512]:
    # row p = stage partition p, un-permuted on the host
    out = nc.dram_tensor("out", [1, 128, 1, H], bf16, kind="ExternalOutput")

    with tile.TileContext(nc) as tc:
        with (
            tc.tile_pool(name="persist", bufs=1) as pp,
            tc.tile_pool(name="psum", bufs=1, space="PSUM") as psp,
        ):
            stage_t = pp.tile([128, 1, 1, H], bf16, tag="stage_t")
            # ---- SBUF tiles ----
            w_b = pp.tile([LC, D], bf16, tag="w_b")
            img_t = [
                [pp.tile([LC, D], bf16, tag=f"img{n}{c}", name=f"img{n}{c}") for c in range(NC)]
                for n in range(NL)
            ]
            # b1's two chunks land in one DMA: [98, (c, 1024)]
            imgCD = pp.tile([LC, NC, D], bf16, tag="imgCD")
            qes_t = pp.tile([128, H], bf16, tag="qes_t")
            s_all = pp.tile([LC, 4], f32, tag="s_all")
            e_bf = pp.tile([LC, 4], bf16, tag="e_bf")
            prodA = pp.tile([LC, D], bf16, tag="prodA")
            dumB = pp.tile([LC, 1], bf16, tag="dumB")
            dumC = pp.tile([LC, 1], bf16, tag="dumC")
            dumD = pp.tile([LC, 1], bf16, tag="dumD")
            pS = [pp.tile([128, 1], f32, tag=f"pS{n}", name=f"pS{n}") for n in range(NL)]
            recip128 = pp.tile([128, 1], f32, tag="recip128")
            cidx = pp.tile([128, 1], mybir.dt.int32, tag="cidx")
            warm = pp.tile([128, H], bf16, tag="warm")

            # ---- PSUM ----
            ps_warm = psp.tile([128, H], f32, tag="ps_warm")
            psA = psp.tile([128, H], f32, tag="psA")

            # preps round-robin onto the DMASW proc lanes in tick order; the
            # completion sem baked into each descriptor must be that lane's
            # canonical sem or the final drain's lane waits never fire
            sw_sems = tc.sems.swdge_block()

            # ---- Pool: kv_writeback prep (descriptor gen hides behind the
            # score phase; the trigger later costs only ~60ns + transfer) ----
            nc.gpsimd.memset(cidx, 0)
            nc.gpsimd.kv_writeback(
                out[:, :, :, :],
                stage_t[:, :, :, :],
                cidx[:, :],
                prepare_only=True,
                sem=sw_sems[0],
            )

            # ---- input DMAs split across the SP and ACT HWDGE queues so
            # the per-DMA sequencer time (~650ns > 558ns transfer) cannot
            # gap the DMA bus; bus order: w, A, B, C, D, qes ----
            nc.sync.dma_start(out=w_b, in_=wb[:, :])
            nc.scalar.dma_start(out=img_t[0][0], in_=img[0, 0, :, :])
            nc.sync.dma_start(out=img_t[0][1], in_=img[0, 1, :, :])
            nc.scalar.dma_start(out=imgCD[:, 0, :], in_=img[1, 0, :, :])
            nc.sync.dma_start(out=imgCD[:, 1, :], in_=img[1, 1, :, :])
            nc.scalar.dma_start(out=qes_t, in_=qes128[:, :])

            # ---- constants + PE clock warmup (keep PE continuously busy:
            # an idle gap resets the p-state ramp) ----
            nc.vector.memset(warm, 0.0)
            nc.vector.memset(pS[0], 0.0)
            nc.vector.memset(pS[1], 0.0)
            for _ in range(12):
                nc.tensor.matmul(ps_warm, warm[:, 0:128], warm, start=True, stop=True)

            # ---- scores ----
            # cols in s_all/e_bf: 0=A(b0c0), 1=B(b0c1), 2=C(b1c0), 3=D(b1c1)
            imgC = imgCD[:, 0, :]
            imgD = imgCD[:, 1, :]
            nc.vector.tensor_mul(prodA, img_t[0][0], w_b)
            nc.scalar.activation(prodA, prodA, Act.Copy, accum_out=s_all[:, 0:1])
            nc.vector.affine_mul_reduce(
                out=dumB.broadcast_to([LC, D]), accum_out=s_all[:, 1:2],
                in0=img_t[0][1], in1=w_b, scale=1.0, bias=0.0,
            )
            # one 2-wide exp for b0: a single ACT op cannot be misordered by
            # the scheduler the way two tiny exps around red-A were
            nc.scalar.activation(e_bf[:, 0:2], s_all[:, 0:2], Act.Exp)

            nc.vector.affine_mul_reduce(
                out=dumC.broadcast_to([LC, D]), accum_out=s_all[:, 2:3],
                in0=imgC, in1=w_b, scale=1.0, bias=0.0,
            )
            nc.vector.affine_mul_reduce(
                out=dumD.broadcast_to([LC, D]), accum_out=s_all[:, 3:4],
                in0=imgD, in1=w_b, scale=1.0, bias=0.0,
            )
            # pin the b1 exps past exp-b0's slot: the scheduler otherwise
            # commits them first and head-of-line blocking in the 4-deep ACT
            # wait queue stalls exp-b0 (and so batch 0's matmuls) by ~1us
            with tc.tile_wait_until(0.0065):
                nc.scalar.activation(e_bf[:, 2:3], s_all[:, 2:3], Act.Exp)
            with tc.tile_wait_until(0.0066):
                nc.scalar.activation(e_bf[:, 3:4], s_all[:, 3:4], Act.Exp)

            # ---- weighted sums into the single psA bank ----
            # batch n: partitions [32n, 32n+32) cols 0:H and [64+32n, 96+32n)
            # cols H:D; the 4 partition regions are independent accumulation
            # groups (the interp's zero-region check ignores partition
            # offsets - regions are truly disjoint, so skip it)
            chunks = [
                [img_t[0][0], img_t[0][1]],
                [imgC, imgD],
            ]

            def emit_att_mms(n):
                c0 = e_bf[:, 2 * n : 2 * n + 1].to_broadcast([LC, T])
                c1 = e_bf[:, 2 * n + 1 : 2 * n + 2].to_broadcast([LC, T])
                lo, hi = 32 * n, 64 + 32 * n
                nc.tensor.matmul(psA[lo : lo + T, :], c0, chunks[n][0][:, 0:H], start=True, stop=False, tile_position=(0, lo), skip_group_check=True)
                nc.tensor.matmul(psA[hi : hi + T, :], c0, chunks[n][0][:, H:D], start=True, stop=False, tile_position=(0, hi), skip_group_check=True)
                nc.tensor.matmul(psA[lo : lo + T, :], c1, chunks[n][1][:, 0:H], start=False, stop=True, tile_position=(0, lo), skip_group_check=True)
                nc.tensor.matmul(psA[hi : hi + T, :], c1, chunks[n][1][:, H:D], start=False, stop=True, tile_position=(0, hi), skip_group_check=True)

            emit_att_mms(0)
            emit_att_mms(1)

            # ---- softmax denominators (off the stt critical path) ----
            # per-batch e-column adds on DVE (free after the AMRs); the
            # partition all-reduces are Pool-only
            for n in range(NL):
                nc.vector.tensor_tensor(
                    out=pS[n][0:LC, :], in0=e_bf[:, 2 * n : 2 * n + 1],
                    in1=e_bf[:, 2 * n + 1 : 2 * n + 2], op=Alu.add,
                )
            for n in range(NL):
                nc.gpsimd.partition_all_reduce(
                    pS[n][:, :], pS[n][:, :], channels=128,
                    reduce_op=bass_isa.ReduceOp.add,
                )
            for n in range(NL):
                lo, hi = 32 * n, 64 + 32 * n
                nc.vector.reciprocal(recip128[lo : lo + T, :], pS[n][lo : lo + T, :])
                nc.vector.reciprocal(recip128[hi : hi + T, :], pS[n][hi : hi + T, :])

            # ---- epilogue: one DVE pass psA*(1/S) + qes -> stage ----
            nc.vector.scalar_tensor_tensor(
                out=stage_t[:, 0, 0, :], in0=psA[:, :], scalar=recip128,
                in1=qes_t[:, :], op0=Alu.mult, op1=Alu.add,
            )

            # ---- fire the prepared output scatter ----
            # Tile's wait-lowering elides the trigger's deferred cross-engine
            # RAW wait on the stt; route it through a tiny Pool read of the
            # stage so the trigger's Pool-tick wait covers it transitively
            # Pool token: its RAW on the stage carries the stt ordering into
            # the Pool stream; the explicit no-sync edge pins the trigger
            # after it (hardware-validated: all 8 cores produce correct
            # output with the transfer ordered after the stt)
            from concourse.instruction_name_ordered_set import (
                InstructionNameOrderedSet,
            )

            tok = pp.tile([1, 1], bf16, tag="tok")
            tok_inst = nc.gpsimd.tensor_copy(tok, stage_t[0:1, 0, 0, 0:1])
            trig = nc.gpsimd.trigger_dma(count=None)
            deps = InstructionNameOrderedSet()
            deps.add(tok_inst.ins.name)
            trig.ins.add_nosync_dependencies_from(deps)

    nc.compile()
    return nc


def _make_in_maps(inputs):
    """Shard the full inputs per core (data-parallel over N, 2 each)."""
    import ml_dtypes

    bf = ml_dtypes.bfloat16
    img_b = np.ascontiguousarray(
        np.asarray(inputs["img_features"], np.float32).reshape(N, NC, LC, D).astype(bf)
    )
    qes_b = np.asarray(inputs["qes_features"], np.float32).astype(bf)
    wb = np.ascontiguousarray(
        np.broadcast_to(np.asarray(inputs["w"], np.float32).astype(bf)[None, :], (LC, D))
    )
    in_maps = []
    for c in range(N_CORES):
        sl = slice(NL * c, NL * (c + 1))
        img_c = img_b[sl].reshape(NL, NC, LC, D)
        q = qes_b[sl]  # [NL, T, D]
        q128 = np.empty((128, H), bf)
        q128[0:32] = q[0, :, 0:H]
        q128[32:64] = q[1, :, 0:H]
        q128[64:96] = q[0, :, H:D]
        q128[96:128] = q[1, :, H:D]
        in_maps.append({"img": img_c, "qes128": np.ascontiguousarray(q128), "wb": wb})
    return in_maps


def kernel(img_features, qes_features, w):
    import os

    os.environ.setdefault("NEURON_RT_RESET_CORES", "1")
    from concourse.bass_utils import run_bass_kernel_spmd

    if "nc" not in _CACHE:
        _CACHE["nc"] = _build_nc()
    nc = _CACHE["nc"]

    in_maps = _make_in_maps(
        {"img_features": img_features, "qes_features": qes_features, "w": w}
    )
    res = run_bass_kernel_spmd(nc, in_maps, core_ids=list(range(N_CORES)))
    outs = []
    for r in res.results:
        o = np.asarray(r["out"], dtype=np.float32).reshape(4, T, H)
        full = np.empty((NL, T, D), np.float32)
        full[0, :, 0:H] = o[0]
        full[1, :, 0:H] = o[1]
        full[0, :, H:D] = o[2]
        full[1, :, H:D] = o[3]
        outs.append(full)
    return np.concatenate(outs, axis=0)
